# revision 57
# baseline (speedup 1.0000x reference)
"""EnsembleActor MLP kernel for Trainium2 (Bass/Tile), expert-parallel over 8 cores.

Math per ensemble head e (E=8, B=4096, OBS=256, H=1024, A=64):
    h1 = relu(x @ W1 + b1)
    h2 = relu(h1 @ W2 + b2)
    mu = h2 @ W3 + b3
    Gs = sum(|mu|, axis=-1)/A ; g = max(Gs, 1)
    mu = mu / g ; pi = mu + 0.1*noise
    return tanh(mu), tanh(pi)

Sharding: one head per NeuronCore (8 heads, 8 cores). Same program on all
cores; per-core inputs differ. No collectives.

Design notes (HW-trace driven; 192.1us baseline -> ~178.7-180us):
- L1/L2 feature-major ([feat, batch]) with bf16 weights stationary, at the
  1 col/cycle PE streaming limit (steady state measures ~19.2us/tile vs
  19.05 theoretical). L3 produces fm[64, BT] (W3 stationary), ACT bias-add
  -> bf16 mu_sb, then ONE xbar dma_start_transpose flips it straight to
  batch-major mu_cat[128, 4, 64] in SBUF (the PE-transpose alternative
  costs ~1.2us/tile of LDW-bound pairs; the flush keeps PE transposes
  because the DMA's ~2us completion latency would sit on the tail).
- Batch-major epilogue, batched per tile: Gs via DVE tensor_reduce(|.|,add)
  per chunk into gs_cat[128,4]; ONE gm/reciprocal pair on [128,4] (the old
  [1,512] serial DVE reciprocal cost 2465ns; this is ~150ns); per-chunk
  mu*r with a per-partition ptr scalar; ONE pi=mu+nz tensor_tensor and ONE
  tanh per output on [128, 256]. Outputs leave as packed 1KB-row DMAs
  (host unpacks); both outputs ride bf16 (the 2e-2 gate has >2.5x margin).
- DMA rings: the scalar(ACT) queue FIFO means every ring instruction placed
  there delays later ACT compute, and HWDGE moves ~2KB-row descriptors at
  ~26-40ns each, so: all DRAM operands are host-packed so each SBUF
  partition's bytes are one contiguous DRAM row; W2 k0-5 halves go per-k on
  sync/scalar (so tile 0's k-sweep can start on k0), k6-7 on the slower
  gpsimd SWDGE ring; noise fully preloads as 2x(64 desc x 8KB).
- Biases ride as row-tensors in ONE 8-descriptor DMA (b3 in row 0's spare
  columns) and get transposed to per-partition layout on the PE during
  startup; the PSUM->SBUF copies run on the DVE so they never queue behind
  ring instructions.
- Warm-up matmuls on iota (varying!) scratch run during the ~7us fixed
  preamble: the clock governor tracks real switching activity, so zero-data
  warm-ups leave the PE at 1.2GHz while iota data ramps it to 2.4GHz in
  ~4us. Long DMA stalls drop the clock again (~1.5us idle decays it), so
  ldweights-only fillers (no PSUM needed - all banks are open during the
  tile-0 k-sweeps) bridge every known startup DMA-wait window.
- Tile 0 uses k-sweep orderings for L1 and L2 (all 8 PSUM groups open
  across ps+mu pools) so compute starts before W1-k1/W2 fully land.
- Last tile: fm in [64,128] column chunks, epilogue in 2-chunk groups with
  per-group output DMAs, pipelined across PE/ACT/DVE for a short tail.
"""

import os
import sys

import numpy as np

for _p in ("/opt/trn_rl_repo", os.path.expanduser("~/.axon_site/_ro/trn_rl_repo")):
    if os.path.isdir(_p) and _p not in sys.path:
        sys.path.insert(0, _p)

E, B, OBS, H, A = 8, 4096, 256, 1024, 64
ACT_NOISE = 0.1
P = 128          # SBUF/PSUM partitions
BT = 512         # batch tile (matmul moving free dim; one PSUM bank fp32)
NBT = B // BT    # 8 batch tiles
KO = OBS // P    # 2 k-chunks in layer 1
KH = H // P      # 8 k-chunks in layers 2/3
NCH = BT // P    # 4 batch-major chunks per tile
BPC = 2 * P + KH + A  # bpack columns: b1 | b2 | id8 | b3(row0)

_PROGRAM = None  # compiled Bacc program cache (one per process)


def _build_program():
    from contextlib import ExitStack

    import concourse.bass as bass
    import concourse.tile as tile
    from concourse import bacc, mybir

    f32 = mybir.dt.float32
    bf16 = mybir.dt.bfloat16
    FT = mybir.ActivationFunctionType
    OP = mybir.AluOpType
    AX = mybir.AxisListType

    nc = bacc.Bacc("TRN2", target_bir_lowering=False, debug=False)

    xpk = nc.dram_tensor("xpk", [P, NBT, KO, BT], bf16, kind="ExternalInput").ap()
    w1pk = nc.dram_tensor("w1pk", [P, KO, H], bf16, kind="ExternalInput").ap()
    w2pk = nc.dram_tensor("w2pk", [P, KH, H], bf16, kind="ExternalInput").ap()
    w3pk = nc.dram_tensor("w3pk", [P, KH, A], bf16, kind="ExternalInput").ap()
    bpk = nc.dram_tensor("bpk", [KH, BPC], f32, kind="ExternalInput").ap()
    id64 = nc.dram_tensor("id64", [A, A], bf16, kind="ExternalInput").ap()
    nzpk = nc.dram_tensor("nzpk", [P, NBT, NCH, A], f32, kind="ExternalInput").ap()
    mupk = nc.dram_tensor("mupk", [P, NBT, NCH, A], bf16, kind="ExternalOutput").ap()
    pipk = nc.dram_tensor("pipk", [P, NBT, NCH, A], bf16, kind="ExternalOutput").ap()

    with tile.TileContext(nc) as tc, ExitStack() as ctx:
        wpool = ctx.enter_context(tc.tile_pool(name="weights", bufs=1))
        xpool = ctx.enter_context(tc.tile_pool(name="x", bufs=3))
        hpool = ctx.enter_context(tc.tile_pool(name="h", bufs=4))
        epool = ctx.enter_context(tc.tile_pool(name="epi", bufs=2))
        opool = ctx.enter_context(tc.tile_pool(name="ostage", bufs=2))
        pspool = ctx.enter_context(tc.tile_pool(name="ps", bufs=4, space="PSUM"))
        mupool = ctx.enter_context(tc.tile_pool(name="mu", bufs=4, space="PSUM"))

        # ---- scratch for PE warm-up (pstate ramp) ----
        # Varying (iota) data, not zeros: the clock governor responds to real
        # switching activity, and constant operands leave the PE looking idle.
        wd = wpool.tile([P, P], bf16, name="wd", tag="wd")
        xd = wpool.tile([P, BT], bf16, name="xd", tag="xd")
        nc.gpsimd.iota(wd[:], [[1, P]], channel_multiplier=3,
                       allow_small_or_imprecise_dtypes=True)
        nc.gpsimd.iota(xd[:], [[1, BT]], channel_multiplier=5,
                       allow_small_or_imprecise_dtypes=True)

        # ---- DMA plan ----
        # sync/scalar (HWDGE): partition-split halves of the critical path:
        # W1 k0, x0, W1 k1, x1, W2 q(0:4), noise, x2, then steady x + outputs.
        # gpsimd/vector (SWDGE): bias rows, W2 q(4:8), W3, identity.
        w1s = wpool.tile([P, KO, H], bf16, name="w1s", tag="w1s")
        xts = {}

        def load_x(bt):
            t = xpool.tile([P, KO, BT], bf16, name=f"xt{bt}", tag="xt")
            nc.sync.dma_start(out=t[0:64, :, :], in_=xpk[0:64, bt, :, :])
            nc.scalar.dma_start(out=t[64:128, :, :], in_=xpk[64:128, bt, :, :])
            xts[bt] = t

        bpkt = wpool.tile([KH, BPC], f32, name="bpkt", tag="bpkt")
        nc.gpsimd.dma_start(out=bpkt[:], in_=bpk[:, :])
        b3mt = bpkt[0:1, 2 * P + KH:]

        nc.sync.dma_start(out=w1s[0:64, 0, :], in_=w1pk[0:64, 0, :])
        nc.scalar.dma_start(out=w1s[64:128, 0, :], in_=w1pk[64:128, 0, :])
        load_x(0)
        nc.sync.dma_start(out=w1s[0:64, 1, :], in_=w1pk[0:64, 1, :])
        nc.scalar.dma_start(out=w1s[64:128, 1, :], in_=w1pk[64:128, 1, :])
        load_x(1)

        # gpsimd's SWDGE ring moves data slower, so it only carries k6-7
        # (needed last); the HWDGE rings carry k0-5 halves, one DMA per k so
        # tile 0's k-sweep layer 2 can start on k0 before the rest lands.
        w2s = wpool.tile([P, KH, H], bf16, name="w2s", tag="w2s")
        wsp = KH - 2
        for k in range(wsp):
            nc.sync.dma_start(out=w2s[0:64, k, :], in_=w2pk[0:64, k, :])
            nc.scalar.dma_start(out=w2s[64:128, k, :], in_=w2pk[64:128, k, :])
        nc.gpsimd.dma_start(out=w2s[0:64, wsp:, :], in_=w2pk[0:64, wsp:, :])
        nc.gpsimd.dma_start(out=w2s[64:128, wsp:, :], in_=w2pk[64:128, wsp:, :])

        w3s = wpool.tile([P, KH, A], bf16, name="w3s", tag="w3s")
        nc.gpsimd.dma_start(out=w3s[:], in_=w3pk[:, :, :])
        id64t = wpool.tile([A, A], bf16, name="id64t", tag="id64t")
        nc.gpsimd.dma_start(out=id64t[:], in_=id64[:, :])

        nzs = wpool.tile([P, NBT, NCH, A], f32, name="nzs", tag="nzs")
        nc.sync.dma_start(out=nzs[0:64, :, :, :], in_=nzpk[0:64, :, :, :])
        nc.scalar.dma_start(out=nzs[64:128, :, :, :], in_=nzpk[64:128, :, :, :])
        load_x(2)

        # ---- PE warm-up while startup DMAs stream ----
        warm = mupool.tile([P, BT], f32, name="warm", tag="mups")
        for _ in range(2):
            nc.tensor.matmul(warm[:, 0:P], lhsT=wd[:], rhs=wd[:],
                             start=True, stop=True)
        for _ in range(8):
            nc.tensor.matmul(warm[:], lhsT=wd[:], rhs=xd[:], start=True, stop=True)

        # ---- bias transposes: row layout -> per-partition columns ----
        # bpkt rows: [8, 0:128]=b1, [8, 128:256]=b2, [8, 256:264]=id8
        b1s = wpool.tile([P, KH], f32, name="b1s", tag="b1s")
        b2s = wpool.tile([P, KH], f32, name="b2s", tag="b2s")
        b3col = wpool.tile([A, 1], f32, name="b3col", tag="b3col")
        id8s = bpkt[:, 2 * P:2 * P + KH]
        for c0, dst, name in ((0, b1s, "b1"), (P, b2s, "b2")):
            ps = mupool.tile([P, KH], f32, name=f"{name}ps", tag="mups")
            nc.tensor.transpose(ps[:], in_=bpkt[:, c0:c0 + P], identity=id8s)
            nc.vector.tensor_scalar(
                out=dst[:], in0=ps[:], scalar1=0.0, scalar2=1.0,
                op0=OP.add, op1=OP.mult)
        ps = mupool.tile([A, 1], f32, name="b3ps", tag="mups")
        nc.tensor.transpose(ps[:], in_=b3mt[:], identity=id8s[0:1, 0:1])
        nc.vector.tensor_scalar(
            out=b3col[:], in0=ps[:], scalar1=0.0, scalar2=1.0,
            op0=OP.add, op1=OP.mult)

        def layer1(bt, first=False):
            """h1 = relu(x @ W1 + b1), feature-major. For bt=0, do k-outer in
            oc-blocks of 4 so compute can start before W1's k1 chunk lands."""
            xt = xts.pop(bt)
            h1s = [None] * KH
            pss = [None] * KH

            def emit_mm(oc, k):
                nc.tensor.matmul(
                    pss[oc][:],
                    lhsT=w1s[:, k, oc * P:(oc + 1) * P],
                    rhs=xt[:, k, :],
                    start=(k == 0),
                    stop=(k == KO - 1),
                )

            def drain(oc):
                h = hpool.tile([P, BT], bf16, name=f"h1_{oc}", tag=f"h1_{oc}")
                if oc % 2 == 0:
                    nc.vector.tensor_scalar(
                        out=h[:], in0=pss[oc][:],
                        scalar1=b1s[:, oc:oc + 1], scalar2=0.0,
                        op0=OP.add, op1=OP.max,
                    )
                else:
                    nc.scalar.activation(
                        out=h[:], in_=pss[oc][:], func=FT.Relu,
                        bias=b1s[:, oc:oc + 1],
                    )
                h1s[oc] = h

            if first:
                # k-sweep: all 8 groups open at once (4 ps + 4 idle mu banks)
                # so the k0 pass runs before W1's k1 chunk even lands
                for oc in range(KH):
                    pool, tag = (pspool, "ps") if oc < 4 else (mupool, "mups")
                    pss[oc] = pool.tile([P, BT], f32, name="ps1", tag=tag)
                for k in range(KO):
                    for oc in range(KH):
                        emit_mm(oc, k)
                    if k == 0:
                        # every PSUM bank is open, so bridge the W1-k1 DMA
                        # wait with ldweights-only fillers: real switching
                        # activity keeps the clock governor from decaying
                        for _ in range(6):
                            nc.tensor.ldweights(weights=wd[:])
                for oc in range(KH):
                    drain(oc)
            else:
                for oc in range(KH):
                    pss[oc] = pspool.tile([P, BT], f32, name="ps1", tag="ps")
                    for k in range(KO):
                        emit_mm(oc, k)
                    drain(oc)
            return h1s

        def layer2(h1s, first=False, fm_sink=None):
            h2s = []

            def emit_fm(k):
                # last tile: fold the fm (h2 @ W3) accumulation into the oc
                # loop, one k behind the relu drains, so no fm work remains
                # after L2 finishes
                nc.tensor.matmul(
                    fm_sink[:], lhsT=w3s[:, k, :], rhs=h2s[k][:],
                    start=(k == 0), stop=(k == KH - 1),
                )

            def drain(oc, ps):
                h = hpool.tile([P, BT], bf16, name=f"h2_{oc}", tag=f"h2_{oc}")
                if oc % 2 == 0:
                    nc.vector.tensor_scalar(
                        out=h[:], in0=ps[:],
                        scalar1=b2s[:, oc:oc + 1], scalar2=0.0,
                        op0=OP.add, op1=OP.max,
                    )
                else:
                    nc.scalar.activation(
                        out=h[:], in_=ps[:], func=FT.Relu,
                        bias=b2s[:, oc:oc + 1],
                    )
                return h

            if first:
                # k-sweep over all 8 output groups: consumes W2 chunk-by-chunk
                # as the per-k startup DMAs land instead of waiting for all
                pss = []
                for oc in range(KH):
                    pool, tag = (pspool, "ps") if oc < 4 else (mupool, "mups")
                    pss.append(pool.tile([P, BT], f32, name="ps2", tag=tag))
                for k in range(KH):
                    for oc in range(KH):
                        nc.tensor.matmul(
                            pss[oc][:],
                            lhsT=w2s[:, k, oc * P:(oc + 1) * P],
                            rhs=h1s[k][:],
                            start=(k == 0),
                            stop=(k == KH - 1),
                        )
                for oc in range(KH):
                    h2s.append(drain(oc, pss[oc]))
                return h2s
            for oc in range(KH):
                ps = pspool.tile([P, BT], f32, name="ps2", tag="ps")
                for k in range(KH):
                    nc.tensor.matmul(
                        ps[:],
                        lhsT=w2s[:, k, oc * P:(oc + 1) * P],
                        rhs=h1s[k][:],
                        start=(k == 0),
                        stop=(k == KH - 1),
                    )
                h2s.append(drain(oc, ps))
                if fm_sink is not None and oc >= 1:
                    emit_fm(oc - 1)
            if fm_sink is not None:
                emit_fm(KH - 1)
            return h2s

        def layer3_fm(h2s, csl=None, cw=BT, on_dve=False):
            """fm[64, cw] = h2 @ W3 in PSUM, then + b3 -> bf16 mu_sb.
            The bias-add runs on ACT normally; the flush rebalances half of
            them onto the DVE (ACT is the flush serializer)."""
            fm = pspool.tile([A, cw], f32, name="fm", tag="ps")
            for k in range(KH):
                rhs = h2s[k][:] if csl is None else h2s[k][:, csl]
                nc.tensor.matmul(
                    fm[:], lhsT=w3s[:, k, :], rhs=rhs,
                    start=(k == 0), stop=(k == KH - 1),
                )
            mu_sb = epool.tile([A, cw], bf16, name="mu_sb", tag="mu_sb")
            if on_dve:
                nc.vector.tensor_scalar(
                    out=mu_sb[:], in0=fm[:], scalar1=b3col[:, 0:1], scalar2=0.0,
                    op0=OP.add, op1=OP.add,
                )
            else:
                nc.scalar.activation(
                    out=mu_sb[:], in_=fm[:], func=FT.Identity,
                    bias=b3col[:, 0:1],
                )
            return mu_sb

        def chunk_front(mu_sb, c0, mu_cat, gs_cat, j):
            """Transpose one [64,128] slice to [128,64], stash bf16 copy and
            per-partition |mu| row-sum."""
            mu_ps = mupool.tile([P, A], bf16, name="mu_ps", tag="mups")
            nc.tensor.transpose(
                mu_ps[:], in_=mu_sb[:, c0:c0 + P], identity=id64t[:])
            nc.vector.tensor_scalar(
                out=mu_cat[:, j, :], in0=mu_ps[:], scalar1=0.0, scalar2=1.0,
                op0=OP.add, op1=OP.mult)
            with nc.allow_low_precision(reason="Gs from bf16 |mu| is plenty"):
                nc.vector.tensor_reduce(
                    out=gs_cat[:, j:j + 1], in_=mu_ps[:], axis=AX.X,
                    op=OP.add, apply_absolute_value=True)

        def epi_back(bt, mu_cat, gs_cat, j0, nj, mu_st, pi_st):
            """Normalize + tanh for chunks j0..j0+nj of tile bt; write staging."""
            jsl = bass.ds(j0, nj)
            gm = epool.tile([P, NCH], f32, name="gm", tag="gm")
            nc.vector.tensor_scalar(
                out=gm[:, jsl], in0=gs_cat[:, jsl], scalar1=1.0 / A, scalar2=1.0,
                op0=OP.mult, op1=OP.max,
            )
            rcol = epool.tile([P, NCH], f32, name="rcol", tag="rcol")
            with nc.allow_low_precision(reason="g==1 exactly for almost all rows"):
                nc.vector.reciprocal(out=rcol[:, jsl], in_=gm[:, jsl])
            mu_n = epool.tile([P, NCH, A], f32, name="mu_n", tag="mu_n")
            for j in range(j0, j0 + nj):
                nc.vector.tensor_scalar(
                    out=mu_n[:, j, :], in0=mu_cat[:, j, :],
                    scalar1=rcol[:, j:j + 1], scalar2=0.0,
                    op0=OP.mult, op1=OP.add,
                )
            pi_n = epool.tile([P, NCH, A], f32, name="pi_n", tag="pi_n")
            nc.vector.tensor_tensor(
                out=pi_n[:, jsl, :], in0=mu_n[:, jsl, :],
                in1=nzs[:, bt, jsl, :], op=OP.add)
            nc.scalar.activation(
                out=mu_st[:, jsl, :], in_=mu_n[:, jsl, :], func=FT.Tanh)
            nc.scalar.activation(
                out=pi_st[:, jsl, :], in_=pi_n[:, jsl, :], func=FT.Tanh)

        def out_dma(bt, mu_st, pi_st, j0, nj, split=False):
            jsl = bass.ds(j0, nj)
            if split:
                # final flush group: halve each output across both rings so
                # the kernel-ending DMA completes sooner
                nc.sync.dma_start(
                    out=mupk[0:64, bt, jsl, :], in_=mu_st[0:64, jsl, :])
                nc.scalar.dma_start(
                    out=mupk[64:128, bt, jsl, :], in_=mu_st[64:128, jsl, :])
                nc.sync.dma_start(
                    out=pipk[0:64, bt, jsl, :], in_=pi_st[0:64, jsl, :])
                nc.scalar.dma_start(
                    out=pipk[64:128, bt, jsl, :], in_=pi_st[64:128, jsl, :])
                return
            nc.sync.dma_start(out=mupk[:, bt, jsl, :], in_=mu_st[:, jsl, :])
            nc.scalar.dma_start(out=pipk[:, bt, jsl, :], in_=pi_st[:, jsl, :])

        # ---- main software pipeline ----
        for _ in range(8):
            nc.tensor.ldweights(weights=wd[:])  # bridge the x0 DMA wait
        h1q = [layer1(0, first=True)]
        for _ in range(6):
            nc.tensor.ldweights(weights=wd[:])  # bridge the x1 DMA wait
        h1q.append(layer1(1))
        for _ in range(10):
            nc.tensor.ldweights(weights=wd[:])  # bridge the W2 tail wait
        for bt in range(NBT):
            if bt + 2 < NBT and bt > 0:
                load_x(bt + 2)
            h2s = layer2(h1q.pop(0), first=(bt == 0))
            mu_st = opool.tile([P, NCH, A], bf16, name="mu_st", tag="mu_st")
            pi_st = opool.tile([P, NCH, A], bf16, name="pi_st", tag="pi_st")
            mu_cat = epool.tile([P, NCH, A], bf16, name="mu_cat", tag="mu_cat")
            gs_cat = epool.tile([P, NCH], f32, name="gs_cat", tag="gs_cat")
            last = bt == NBT - 1
            if not last:
                mu_sb = layer3_fm(h2s)
                if bt + 2 < NBT:
                    h1q.append(layer1(bt + 2))
                # xbar DMA transpose straight to batch-major SBUF: frees the
                # PE of 4 transposes/tile (~1.2us of LDW-bound pairs) and the
                # DVE of the per-chunk copies; Gs is then ONE 3D reduce
                nc.sync.dma_start_transpose(out=mu_cat[:, :, :], in_=mu_sb[:])
                with nc.allow_low_precision(reason="Gs from bf16 |mu|"):
                    nc.vector.tensor_reduce(
                        out=gs_cat[:, :], in_=mu_cat[:, :, :], axis=AX.X,
                        op=OP.add, apply_absolute_value=True)
                epi_back(bt, mu_cat, gs_cat, 0, NCH, mu_st, pi_st)
                out_dma(bt, mu_st, pi_st, 0, NCH)
            else:
                # chunked flush: 2-chunk epilogue groups, early output DMAs
                mu_sbs = [None] * NCH
                for j in range(NCH):
                    mu_sbs[j] = layer3_fm(
                        h2s, csl=bass.ds(j * P, P), cw=P, on_dve=(j % 2 == 0))
                    if j >= 1:
                        chunk_front(mu_sbs[j - 1], 0, mu_cat, gs_cat, j - 1)
                    if j == 2:
                        epi_back(bt, mu_cat, gs_cat, 0, 2, mu_st, pi_st)
                        out_dma(bt, mu_st, pi_st, 0, 2)
                chunk_front(mu_sbs[NCH - 1], 0, mu_cat, gs_cat, NCH - 1)
                epi_back(bt, mu_cat, gs_cat, 2, 2, mu_st, pi_st)
                out_dma(bt, mu_st, pi_st, 2, 2)

    nc.compile()
    return nc


def _get_program():
    global _PROGRAM
    if _PROGRAM is None:
        _PROGRAM = _build_program()
    return _PROGRAM


def run(inputs, trace=False, trace_cores=None, tmpdir=None):
    """Returns (outputs_tuple, BassKernelResults)."""
    import ml_dtypes

    from concourse.bass_utils import run_bass_kernel_spmd

    nc = _get_program()
    bf = ml_dtypes.bfloat16

    x = np.asarray(inputs["x"], dtype=np.float32)
    noise = np.asarray(inputs["noise"], dtype=np.float32)
    W1 = np.asarray(inputs["W1"], dtype=np.float32)
    b1 = np.asarray(inputs["b1"], dtype=np.float32)
    W2 = np.asarray(inputs["W2"], dtype=np.float32)
    b2 = np.asarray(inputs["b2"], dtype=np.float32)
    W3 = np.asarray(inputs["W3"], dtype=np.float32)
    b3 = np.asarray(inputs["b3"], dtype=np.float32)

    in_maps = []
    for e in range(E):
        xT = x[e].T  # [OBS, B]
        xpk = np.ascontiguousarray(
            xT.reshape(KO, P, NBT, BT).transpose(1, 2, 0, 3).astype(bf))
        w1pk = np.ascontiguousarray(
            W1[e].reshape(KO, P, H).transpose(1, 0, 2).astype(bf))
        w2pk = np.ascontiguousarray(
            W2[e].reshape(KH, P, H).transpose(1, 0, 2).astype(bf))
        w3pk = np.ascontiguousarray(
            W3[e].reshape(KH, P, A).transpose(1, 0, 2).astype(bf))
        bpk = np.zeros((KH, BPC), dtype=np.float32)
        bpk[:, 0:P] = b1[e].reshape(KH, P)
        bpk[:, P:2 * P] = b2[e].reshape(KH, P)
        bpk[:, 2 * P:2 * P + KH] = np.eye(KH, dtype=np.float32)
        bpk[0, 2 * P + KH:] = b3[e]
        nz = (ACT_NOISE * noise[e]).reshape(NBT, NCH, P, A)
        nzpk = np.ascontiguousarray(nz.transpose(2, 0, 1, 3))
        in_maps.append({
            "xpk": xpk,
            "w1pk": w1pk,
            "w2pk": w2pk,
            "w3pk": w3pk,
            "bpk": bpk,
            "id64": np.eye(A, dtype=bf),
            "nzpk": nzpk,
        })

    res = run_bass_kernel_spmd(
        nc, in_maps, core_ids=list(range(E)), trace=trace,
        trace_cores=trace_cores, tmpdir=tmpdir,
    )

    def unpack(r, name):
        # [P, NBT, NCH, A] -> [B, A]
        return r[name].astype(np.float32).transpose(1, 2, 0, 3).reshape(B, A)

    mu = np.stack([unpack(res.results[e], "mupk") for e in range(E)])
    pi = np.stack([unpack(res.results[e], "pipk") for e in range(E)])
    return (np.ascontiguousarray(mu), np.ascontiguousarray(pi)), res


def kernel(**inputs):
    outs, _ = run(inputs, trace=False)
    return outs


# revision 59
# speedup vs baseline: 1.0083x; 1.0083x over previous
"""EnsembleActor MLP kernel for Trainium2 (Bass/Tile), expert-parallel over 8 cores.

Math per ensemble head e (E=8, B=4096, OBS=256, H=1024, A=64):
    h1 = relu(x @ W1 + b1)
    h2 = relu(h1 @ W2 + b2)
    mu = h2 @ W3 + b3
    Gs = sum(|mu|, axis=-1)/A ; g = max(Gs, 1)
    mu = mu / g ; pi = mu + 0.1*noise
    return tanh(mu), tanh(pi)

Sharding: one head per NeuronCore (8 heads, 8 cores). Same program on all
cores; per-core inputs differ. No collectives.

Design notes (HW-trace driven; 192.1us baseline -> ~178.7-180us):
- L1/L2 feature-major ([feat, batch]) with bf16 weights stationary, at the
  1 col/cycle PE streaming limit (steady state measures ~19.2us/tile vs
  19.05 theoretical). L3 produces fm[64, BT] (W3 stationary), ACT bias-add
  -> bf16 mu_sb, then ONE xbar dma_start_transpose flips it straight to
  batch-major mu_cat[128, 4, 64] in SBUF (the PE-transpose alternative
  costs ~1.2us/tile of LDW-bound pairs; the flush keeps PE transposes
  because the DMA's ~2us completion latency would sit on the tail).
- Batch-major epilogue, batched per tile: Gs via DVE tensor_reduce(|.|,add)
  per chunk into gs_cat[128,4]; ONE gm/reciprocal pair on [128,4] (the old
  [1,512] serial DVE reciprocal cost 2465ns; this is ~150ns); per-chunk
  mu*r with a per-partition ptr scalar; ONE pi=mu+nz tensor_tensor and ONE
  tanh per output on [128, 256]. Outputs leave as packed 1KB-row DMAs
  (host unpacks); both outputs ride bf16 (the 2e-2 gate has >2.5x margin).
- DMA rings: the scalar(ACT) queue FIFO means every ring instruction placed
  there delays later ACT compute, and HWDGE moves ~2KB-row descriptors at
  ~26-40ns each, so: all DRAM operands are host-packed so each SBUF
  partition's bytes are one contiguous DRAM row; W2 k0-5 halves go per-k on
  sync/scalar (so tile 0's k-sweep can start on k0), k6-7 on the slower
  gpsimd SWDGE ring; noise fully preloads as 2x(64 desc x 8KB).
- Biases ride as row-tensors in ONE 8-descriptor DMA (b3 in row 0's spare
  columns) and get transposed to per-partition layout on the PE during
  startup; the PSUM->SBUF copies run on the DVE so they never queue behind
  ring instructions.
- Warm-up matmuls on iota (varying!) scratch run during the ~7us fixed
  preamble: the clock governor tracks real switching activity, so zero-data
  warm-ups leave the PE at 1.2GHz while iota data ramps it to 2.4GHz in
  ~4us. Long DMA stalls drop the clock again (~1.5us idle decays it), so
  ldweights-only fillers (no PSUM needed - all banks are open during the
  tile-0 k-sweeps) bridge every known startup DMA-wait window.
- Tile 0 uses k-sweep orderings for L1 and L2 (all 8 PSUM groups open
  across ps+mu pools) so compute starts before W1-k1/W2 fully land.
- Last tile: fm in [64,128] column chunks, epilogue in 2-chunk groups with
  per-group output DMAs, pipelined across PE/ACT/DVE for a short tail.
"""

import os
import sys

import numpy as np

for _p in ("/opt/trn_rl_repo", os.path.expanduser("~/.axon_site/_ro/trn_rl_repo")):
    if os.path.isdir(_p) and _p not in sys.path:
        sys.path.insert(0, _p)

E, B, OBS, H, A = 8, 4096, 256, 1024, 64
ACT_NOISE = 0.1
P = 128          # SBUF/PSUM partitions
BT = 512         # batch tile (matmul moving free dim; one PSUM bank fp32)
NBT = B // BT    # 8 batch tiles
KO = OBS // P    # 2 k-chunks in layer 1
KH = H // P      # 8 k-chunks in layers 2/3
NCH = BT // P    # 4 batch-major chunks per tile
BPC = 2 * P + KH + A  # bpack columns: b1 | b2 | id8 | b3(row0)

_PROGRAM = None  # compiled Bacc program cache (one per process)


def _build_program():
    from contextlib import ExitStack

    import concourse.bass as bass
    import concourse.tile as tile
    from concourse import bacc, mybir

    f32 = mybir.dt.float32
    bf16 = mybir.dt.bfloat16
    FT = mybir.ActivationFunctionType
    OP = mybir.AluOpType
    AX = mybir.AxisListType

    nc = bacc.Bacc("TRN2", target_bir_lowering=False, debug=False)

    xpk = nc.dram_tensor("xpk", [P, NBT, KO, BT], bf16, kind="ExternalInput").ap()
    w1pk = nc.dram_tensor("w1pk", [P, KO, H], bf16, kind="ExternalInput").ap()
    w2pk = nc.dram_tensor("w2pk", [P, KH, H], bf16, kind="ExternalInput").ap()
    w3pk = nc.dram_tensor("w3pk", [P, KH, A], bf16, kind="ExternalInput").ap()
    bpk = nc.dram_tensor("bpk", [KH, BPC], f32, kind="ExternalInput").ap()
    id64 = nc.dram_tensor("id64", [A, A], bf16, kind="ExternalInput").ap()
    nzpk = nc.dram_tensor("nzpk", [P, NBT, NCH, A], f32, kind="ExternalInput").ap()
    mupk = nc.dram_tensor("mupk", [P, NBT, NCH, A], bf16, kind="ExternalOutput").ap()
    pipk = nc.dram_tensor("pipk", [P, NBT, NCH, A], bf16, kind="ExternalOutput").ap()

    with tile.TileContext(nc) as tc, ExitStack() as ctx:
        wpool = ctx.enter_context(tc.tile_pool(name="weights", bufs=1))
        xpool = ctx.enter_context(tc.tile_pool(name="x", bufs=3))
        hpool = ctx.enter_context(tc.tile_pool(name="h", bufs=4))
        epool = ctx.enter_context(tc.tile_pool(name="epi", bufs=2))
        opool = ctx.enter_context(tc.tile_pool(name="ostage", bufs=2))
        pspool = ctx.enter_context(tc.tile_pool(name="ps", bufs=4, space="PSUM"))
        mupool = ctx.enter_context(tc.tile_pool(name="mu", bufs=4, space="PSUM"))

        # ---- scratch for PE warm-up (pstate ramp) ----
        # Varying (iota) data, not zeros: the clock governor responds to real
        # switching activity, and constant operands leave the PE looking idle.
        wd = wpool.tile([P, P], bf16, name="wd", tag="wd")
        xd = wpool.tile([P, BT], bf16, name="xd", tag="xd")
        nc.gpsimd.iota(wd[:], [[1, P]], channel_multiplier=3,
                       allow_small_or_imprecise_dtypes=True)
        nc.gpsimd.iota(xd[:], [[1, BT]], channel_multiplier=5,
                       allow_small_or_imprecise_dtypes=True)

        # ---- DMA plan ----
        # sync/scalar (HWDGE): partition-split halves of the critical path:
        # W1 k0, x0, W1 k1, x1, W2 q(0:4), noise, x2, then steady x + outputs.
        # gpsimd/vector (SWDGE): bias rows, W2 q(4:8), W3, identity.
        w1s = wpool.tile([P, KO, H], bf16, name="w1s", tag="w1s")
        xts = {}

        def load_x(bt):
            t = xpool.tile([P, KO, BT], bf16, name=f"xt{bt}", tag="xt")
            nc.sync.dma_start(out=t[0:64, :, :], in_=xpk[0:64, bt, :, :])
            nc.scalar.dma_start(out=t[64:128, :, :], in_=xpk[64:128, bt, :, :])
            xts[bt] = t

        bpkt = wpool.tile([KH, BPC], f32, name="bpkt", tag="bpkt")
        nc.gpsimd.dma_start(out=bpkt[:], in_=bpk[:, :])
        b3mt = bpkt[0:1, 2 * P + KH:]

        nc.sync.dma_start(out=w1s[0:64, 0, :], in_=w1pk[0:64, 0, :])
        nc.scalar.dma_start(out=w1s[64:128, 0, :], in_=w1pk[64:128, 0, :])
        load_x(0)
        nc.sync.dma_start(out=w1s[0:64, 1, :], in_=w1pk[0:64, 1, :])
        nc.scalar.dma_start(out=w1s[64:128, 1, :], in_=w1pk[64:128, 1, :])
        load_x(1)

        # gpsimd's SWDGE ring moves data slower, so it only carries k6-7
        # (needed last); the HWDGE rings carry k0-5 halves, one DMA per k so
        # tile 0's k-sweep layer 2 can start on k0 before the rest lands.
        w2s = wpool.tile([P, KH, H], bf16, name="w2s", tag="w2s")
        wsp = KH - 2
        for k in range(wsp):
            nc.sync.dma_start(out=w2s[0:64, k, :], in_=w2pk[0:64, k, :])
            nc.scalar.dma_start(out=w2s[64:128, k, :], in_=w2pk[64:128, k, :])
        nc.gpsimd.dma_start(out=w2s[0:64, wsp:, :], in_=w2pk[0:64, wsp:, :])
        nc.gpsimd.dma_start(out=w2s[64:128, wsp:, :], in_=w2pk[64:128, wsp:, :])

        w3s = wpool.tile([P, KH, A], bf16, name="w3s", tag="w3s")
        nc.gpsimd.dma_start(out=w3s[:], in_=w3pk[:, :, :])
        id64t = wpool.tile([A, A], bf16, name="id64t", tag="id64t")
        nc.gpsimd.dma_start(out=id64t[:], in_=id64[:, :])

        nzs = wpool.tile([P, NBT, NCH, A], f32, name="nzs", tag="nzs")
        nc.sync.dma_start(out=nzs[0:64, :, :, :], in_=nzpk[0:64, :, :, :])
        nc.scalar.dma_start(out=nzs[64:128, :, :, :], in_=nzpk[64:128, :, :, :])
        load_x(2)

        # ---- PE warm-up while startup DMAs stream ----
        warm = mupool.tile([P, BT], f32, name="warm", tag="mups")
        for _ in range(2):
            nc.tensor.matmul(warm[:, 0:P], lhsT=wd[:], rhs=wd[:],
                             start=True, stop=True)
        for _ in range(8):
            nc.tensor.matmul(warm[:], lhsT=wd[:], rhs=xd[:], start=True, stop=True)

        # ---- bias transposes: row layout -> per-partition columns ----
        # bpkt rows: [8, 0:128]=b1, [8, 128:256]=b2, [8, 256:264]=id8
        b1s = wpool.tile([P, KH], f32, name="b1s", tag="b1s")
        b2s = wpool.tile([P, KH], f32, name="b2s", tag="b2s")
        b3col = wpool.tile([A, 1], f32, name="b3col", tag="b3col")
        id8s = bpkt[:, 2 * P:2 * P + KH]
        for c0, dst, name in ((0, b1s, "b1"), (P, b2s, "b2")):
            ps = mupool.tile([P, KH], f32, name=f"{name}ps", tag="mups")
            nc.tensor.transpose(ps[:], in_=bpkt[:, c0:c0 + P], identity=id8s)
            nc.vector.tensor_scalar(
                out=dst[:], in0=ps[:], scalar1=0.0, scalar2=1.0,
                op0=OP.add, op1=OP.mult)
        ps = mupool.tile([A, 1], f32, name="b3ps", tag="mups")
        nc.tensor.transpose(ps[:], in_=b3mt[:], identity=id8s[0:1, 0:1])
        nc.vector.tensor_scalar(
            out=b3col[:], in0=ps[:], scalar1=0.0, scalar2=1.0,
            op0=OP.add, op1=OP.mult)

        def layer1(bt, first=False):
            """h1 = relu(x @ W1 + b1), feature-major. For bt=0, do k-outer in
            oc-blocks of 4 so compute can start before W1's k1 chunk lands."""
            xt = xts.pop(bt)
            h1s = [None] * KH
            pss = [None] * KH

            def emit_mm(oc, k):
                nc.tensor.matmul(
                    pss[oc][:],
                    lhsT=w1s[:, k, oc * P:(oc + 1) * P],
                    rhs=xt[:, k, :],
                    start=(k == 0),
                    stop=(k == KO - 1),
                )

            def drain(oc):
                h = hpool.tile([P, BT], bf16, name=f"h1_{oc}", tag=f"h1_{oc}")
                if oc % 2 == 0:
                    nc.vector.tensor_scalar(
                        out=h[:], in0=pss[oc][:],
                        scalar1=b1s[:, oc:oc + 1], scalar2=0.0,
                        op0=OP.add, op1=OP.max,
                    )
                else:
                    nc.scalar.activation(
                        out=h[:], in_=pss[oc][:], func=FT.Relu,
                        bias=b1s[:, oc:oc + 1],
                    )
                h1s[oc] = h

            if first:
                # k-sweep: all 8 groups open at once (4 ps + 4 idle mu banks)
                # so the k0 pass runs before W1's k1 chunk even lands
                for oc in range(KH):
                    pool, tag = (pspool, "ps") if oc < 4 else (mupool, "mups")
                    pss[oc] = pool.tile([P, BT], f32, name="ps1", tag=tag)
                for k in range(KO):
                    for oc in range(KH):
                        emit_mm(oc, k)
                    if k == 0:
                        # every PSUM bank is open, so bridge the W1-k1 DMA
                        # wait with ldweights-only fillers: real switching
                        # activity keeps the clock governor from decaying
                        for _ in range(6):
                            nc.tensor.ldweights(weights=wd[:])
                for oc in range(KH):
                    drain(oc)
            else:
                for oc in range(KH):
                    pss[oc] = pspool.tile([P, BT], f32, name="ps1", tag="ps")
                    for k in range(KO):
                        emit_mm(oc, k)
                    drain(oc)
            return h1s

        def layer2(h1s, first=False, fm_sink=None):
            h2s = []

            def emit_fm(k):
                # last tile: fold the fm (h2 @ W3) accumulation into the oc
                # loop, one k behind the relu drains, so no fm work remains
                # after L2 finishes
                nc.tensor.matmul(
                    fm_sink[:], lhsT=w3s[:, k, :], rhs=h2s[k][:],
                    start=(k == 0), stop=(k == KH - 1),
                )

            def drain(oc, ps):
                h = hpool.tile([P, BT], bf16, name=f"h2_{oc}", tag=f"h2_{oc}")
                if oc % 2 == 0:
                    nc.vector.tensor_scalar(
                        out=h[:], in0=ps[:],
                        scalar1=b2s[:, oc:oc + 1], scalar2=0.0,
                        op0=OP.add, op1=OP.max,
                    )
                else:
                    nc.scalar.activation(
                        out=h[:], in_=ps[:], func=FT.Relu,
                        bias=b2s[:, oc:oc + 1],
                    )
                return h

            if first:
                # k-sweep over all 8 output groups: consumes W2 chunk-by-chunk
                # as the per-k startup DMAs land instead of waiting for all
                pss = []
                for oc in range(KH):
                    pool, tag = (pspool, "ps") if oc < 4 else (mupool, "mups")
                    pss.append(pool.tile([P, BT], f32, name="ps2", tag=tag))
                for k in range(KH):
                    for oc in range(KH):
                        nc.tensor.matmul(
                            pss[oc][:],
                            lhsT=w2s[:, k, oc * P:(oc + 1) * P],
                            rhs=h1s[k][:],
                            start=(k == 0),
                            stop=(k == KH - 1),
                        )
                for oc in range(KH):
                    h2s.append(drain(oc, pss[oc]))
                return h2s
            for oc in range(KH):
                ps = pspool.tile([P, BT], f32, name="ps2", tag="ps")
                for k in range(KH):
                    nc.tensor.matmul(
                        ps[:],
                        lhsT=w2s[:, k, oc * P:(oc + 1) * P],
                        rhs=h1s[k][:],
                        start=(k == 0),
                        stop=(k == KH - 1),
                    )
                h2s.append(drain(oc, ps))
                if fm_sink is not None and oc >= 1:
                    emit_fm(oc - 1)
            if fm_sink is not None:
                emit_fm(KH - 1)
            return h2s

        def layer3_fm(h2s, csl=None, cw=BT, on_dve=False):
            """fm[64, cw] = h2 @ W3 in PSUM, then + b3 -> bf16 mu_sb.
            The bias-add runs on ACT normally; the flush rebalances half of
            them onto the DVE (ACT is the flush serializer)."""
            fm = pspool.tile([A, cw], f32, name="fm", tag="ps")
            for k in range(KH):
                rhs = h2s[k][:] if csl is None else h2s[k][:, csl]
                nc.tensor.matmul(
                    fm[:], lhsT=w3s[:, k, :], rhs=rhs,
                    start=(k == 0), stop=(k == KH - 1),
                )
            mu_sb = epool.tile([A, cw], bf16, name="mu_sb", tag="mu_sb")
            if on_dve:
                nc.vector.tensor_scalar(
                    out=mu_sb[:], in0=fm[:], scalar1=b3col[:, 0:1], scalar2=0.0,
                    op0=OP.add, op1=OP.add,
                )
            else:
                nc.scalar.activation(
                    out=mu_sb[:], in_=fm[:], func=FT.Identity,
                    bias=b3col[:, 0:1],
                )
            return mu_sb

        def chunk_front(mu_sb, c0, mu_cat, gs_cat, j):
            """Transpose one [64,128] slice to [128,64], stash bf16 copy and
            per-partition |mu| row-sum."""
            mu_ps = mupool.tile([P, A], bf16, name="mu_ps", tag="mups")
            nc.tensor.transpose(
                mu_ps[:], in_=mu_sb[:, c0:c0 + P], identity=id64t[:])
            nc.vector.tensor_scalar(
                out=mu_cat[:, j, :], in0=mu_ps[:], scalar1=0.0, scalar2=1.0,
                op0=OP.add, op1=OP.mult)
            with nc.allow_low_precision(reason="Gs from bf16 |mu| is plenty"):
                nc.vector.tensor_reduce(
                    out=gs_cat[:, j:j + 1], in_=mu_ps[:], axis=AX.X,
                    op=OP.add, apply_absolute_value=True)

        def epi_back(bt, mu_cat, gs_cat, j0, nj, mu_st, pi_st):
            """Normalize + tanh for chunks j0..j0+nj of tile bt; write staging."""
            jsl = bass.ds(j0, nj)
            gm = epool.tile([P, NCH], f32, name="gm", tag="gm")
            nc.vector.tensor_scalar(
                out=gm[:, jsl], in0=gs_cat[:, jsl], scalar1=1.0 / A, scalar2=1.0,
                op0=OP.mult, op1=OP.max,
            )
            rcol = epool.tile([P, NCH], f32, name="rcol", tag="rcol")
            with nc.allow_low_precision(reason="g==1 exactly for almost all rows"):
                nc.vector.reciprocal(out=rcol[:, jsl], in_=gm[:, jsl])
            mu_n = epool.tile([P, NCH, A], f32, name="mu_n", tag="mu_n")
            for j in range(j0, j0 + nj):
                nc.vector.tensor_scalar(
                    out=mu_n[:, j, :], in0=mu_cat[:, j, :],
                    scalar1=rcol[:, j:j + 1], scalar2=0.0,
                    op0=OP.mult, op1=OP.add,
                )
            pi_n = epool.tile([P, NCH, A], f32, name="pi_n", tag="pi_n")
            nc.vector.tensor_tensor(
                out=pi_n[:, jsl, :], in0=mu_n[:, jsl, :],
                in1=nzs[:, bt, jsl, :], op=OP.add)
            nc.scalar.activation(
                out=mu_st[:, jsl, :], in_=mu_n[:, jsl, :], func=FT.Tanh)
            nc.scalar.activation(
                out=pi_st[:, jsl, :], in_=pi_n[:, jsl, :], func=FT.Tanh)

        def out_dma(bt, mu_st, pi_st, j0, nj, split=False):
            jsl = bass.ds(j0, nj)
            if split:
                # final flush group: halve each output across both rings so
                # the kernel-ending DMA completes sooner
                nc.sync.dma_start(
                    out=mupk[0:64, bt, jsl, :], in_=mu_st[0:64, jsl, :])
                nc.scalar.dma_start(
                    out=mupk[64:128, bt, jsl, :], in_=mu_st[64:128, jsl, :])
                nc.sync.dma_start(
                    out=pipk[0:64, bt, jsl, :], in_=pi_st[0:64, jsl, :])
                nc.scalar.dma_start(
                    out=pipk[64:128, bt, jsl, :], in_=pi_st[64:128, jsl, :])
                return
            nc.sync.dma_start(out=mupk[:, bt, jsl, :], in_=mu_st[:, jsl, :])
            nc.scalar.dma_start(out=pipk[:, bt, jsl, :], in_=pi_st[:, jsl, :])

        # ---- main software pipeline ----
        for _ in range(8):
            nc.tensor.ldweights(weights=wd[:])  # bridge the x0 DMA wait
        h1q = [layer1(0, first=True)]
        for _ in range(6):
            nc.tensor.ldweights(weights=wd[:])  # bridge the x1 DMA wait
        h1q.append(layer1(1))
        for _ in range(10):
            nc.tensor.ldweights(weights=wd[:])  # bridge the W2 tail wait
        for bt in range(NBT):
            if bt + 2 < NBT and bt > 0:
                load_x(bt + 2)
            h2s = layer2(h1q.pop(0), first=(bt == 0))
            mu_st = opool.tile([P, NCH, A], bf16, name="mu_st", tag="mu_st")
            pi_st = opool.tile([P, NCH, A], bf16, name="pi_st", tag="pi_st")
            mu_cat = epool.tile([P, NCH, A], bf16, name="mu_cat", tag="mu_cat")
            gs_cat = epool.tile([P, NCH], f32, name="gs_cat", tag="gs_cat")
            last = bt == NBT - 1
            if not last:
                mu_sb = layer3_fm(h2s)
                if bt + 2 < NBT:
                    h1q.append(layer1(bt + 2))
                # xbar DMA transpose straight to batch-major SBUF: frees the
                # PE of 4 transposes/tile (~1.2us of LDW-bound pairs) and the
                # DVE of the per-chunk copies; Gs is then ONE 3D reduce
                nc.sync.dma_start_transpose(out=mu_cat[:, :, :], in_=mu_sb[:])
                with nc.allow_low_precision(reason="Gs from bf16 |mu|"):
                    nc.vector.tensor_reduce(
                        out=gs_cat[:, :], in_=mu_cat[:, :, :], axis=AX.X,
                        op=OP.add, apply_absolute_value=True)
                epi_back(bt, mu_cat, gs_cat, 0, NCH, mu_st, pi_st)
                out_dma(bt, mu_st, pi_st, 0, NCH)
            else:
                # chunked flush: 2-chunk epilogue groups, early output DMAs
                mu_sbs = [None] * NCH
                for j in range(NCH):
                    mu_sbs[j] = layer3_fm(
                        h2s, csl=bass.ds(j * P, P), cw=P, on_dve=(j % 2 == 0))
                    if j >= 1:
                        chunk_front(mu_sbs[j - 1], 0, mu_cat, gs_cat, j - 1)
                    if j == 2:
                        epi_back(bt, mu_cat, gs_cat, 0, 2, mu_st, pi_st)
                        out_dma(bt, mu_st, pi_st, 0, 2)
                chunk_front(mu_sbs[NCH - 1], 0, mu_cat, gs_cat, NCH - 1)
                epi_back(bt, mu_cat, gs_cat, 2, 2, mu_st, pi_st)
                out_dma(bt, mu_st, pi_st, 2, 2)

    nc.compile()
    return nc


def _get_program():
    global _PROGRAM
    if _PROGRAM is None:
        _PROGRAM = _build_program()
    return _PROGRAM


def run(inputs, trace=False, trace_cores=None, tmpdir=None):
    """Returns (outputs_tuple, BassKernelResults)."""
    import ml_dtypes

    from concourse.bass_utils import run_bass_kernel_spmd

    nc = _get_program()
    bf = ml_dtypes.bfloat16

    x = np.asarray(inputs["x"], dtype=np.float32)
    noise = np.asarray(inputs["noise"], dtype=np.float32)
    W1 = np.asarray(inputs["W1"], dtype=np.float32)
    b1 = np.asarray(inputs["b1"], dtype=np.float32)
    W2 = np.asarray(inputs["W2"], dtype=np.float32)
    b2 = np.asarray(inputs["b2"], dtype=np.float32)
    W3 = np.asarray(inputs["W3"], dtype=np.float32)
    b3 = np.asarray(inputs["b3"], dtype=np.float32)

    in_maps = []
    for e in range(E):
        xT = x[e].T  # [OBS, B]
        xpk = np.ascontiguousarray(
            xT.reshape(KO, P, NBT, BT).transpose(1, 2, 0, 3).astype(bf))
        w1pk = np.ascontiguousarray(
            W1[e].reshape(KO, P, H).transpose(1, 0, 2).astype(bf))
        w2pk = np.ascontiguousarray(
            W2[e].reshape(KH, P, H).transpose(1, 0, 2).astype(bf))
        w3pk = np.ascontiguousarray(
            W3[e].reshape(KH, P, A).transpose(1, 0, 2).astype(bf))
        bpk = np.zeros((KH, BPC), dtype=np.float32)
        bpk[:, 0:P] = b1[e].reshape(KH, P)
        bpk[:, P:2 * P] = b2[e].reshape(KH, P)
        bpk[:, 2 * P:2 * P + KH] = np.eye(KH, dtype=np.float32)
        bpk[0, 2 * P + KH:] = b3[e]
        nz = (ACT_NOISE * noise[e]).reshape(NBT, NCH, P, A)
        nzpk = np.ascontiguousarray(nz.transpose(2, 0, 1, 3))
        in_maps.append({
            "xpk": xpk,
            "w1pk": w1pk,
            "w2pk": w2pk,
            "w3pk": w3pk,
            "bpk": bpk,
            "id64": np.eye(A, dtype=bf),
            "nzpk": nzpk,
        })

    res = run_bass_kernel_spmd(
        nc, in_maps, core_ids=list(range(E)), trace=trace,
        trace_cores=trace_cores, tmpdir=tmpdir,
    )

    def unpack(r, name):
        # [P, NBT, NCH, A] -> [B, A]
        return r[name].astype(np.float32).transpose(1, 2, 0, 3).reshape(B, A)

    mu = np.stack([unpack(res.results[e], "mupk") for e in range(E)])
    pi = np.stack([unpack(res.results[e], "pipk") for e in range(E)])
    return (np.ascontiguousarray(mu), np.ascontiguousarray(pi)), res


def kernel(**inputs):
    outs, _ = run(inputs, trace=False)
    return outs
